# revision 12
# baseline (speedup 1.0000x reference)
"""Trainium2 Bass kernel: grouped similarity-gating normalization.

Reference computation (per batch b, group g, cpg=64 channels, hw=784):
    means[c]  = mean_hw(x[c, :])
    s[hw]     = sum_c x[c, hw] * means[c]
    t         = (s - mean(s)) * rsqrt(var(s) + eps)
    gate      = sigmoid(t * weight[g] + bias[g])
    out[c,hw] = x[c, hw] * gate[hw]

Sharding: data-parallel over batch B=64 across 8 cores (8 batches/core).

Per-core layout: one SBUF tile [128, 4, 784] per batch holds channels
c = 4*p + j (p = partition, j = free chunk) -> contiguous 1.6MB DMAs, and
group(c) = c//64 = p//16, i.e. each group owns a 16-partition band.

v2 design (memory-regime; HBM floor ~54us/core with fp16 output):
  - input DMAs ride the scalar (ACT) HWDGE ring, output DMAs the sync
    (SP) ring -> no head-of-line blocking between loads and stores.
  - channel sums: j0 via one DVE reduce, j1-3 via ACT Copy+accum_out
    (copy lives in the sigmoid table set -> no ACT table swaps at all).
  - s via PE: 4 accumulating fp32r matmuls with lhsT = indicator*means.
  - mean/var of s via DVE bn_stats/bn_aggr straight from PSUM (exact:
    4 equal-count sub-groups of 196).
  - rstd on DVE: int32 magic-constant seed (shift/xor/add) + 3 Newton
    iterations -> no Sqrt on ACT, so zero ACT_TABLE_LOADs in steady
    state (all ACT funcs sit in sigmoid_and_others).
  - gate = sigmoid(s*a + c) one ACT op with per-partition scale/bias.
  - gating multiply split DVE (j0,j1) / GpSimd (j2,j3), each as ONE
    broadcast-middle-dim tensor_tensor writing fp16 -> output HBM
    traffic halved; host upcasts to fp32 (rel err ~4e-4 << 2e-2 gate).
"""

import sys

if "/opt/trn_rl_repo" not in sys.path:
    sys.path.insert(0, "/opt/trn_rl_repo")

from contextlib import ExitStack

import numpy as np

import concourse.bacc as bacc
import concourse.tile as tile
from concourse import mybir
from concourse.bass_utils import run_bass_kernel_spmd

B, C, H, W = 64, 512, 28, 28
G = 8
HW = H * W          # 784
NCORES = 8
BLOC = B // NCORES  # 8 batches per core
NP = 128            # SBUF partitions
NJ = C // NP        # 4 channel chunks per partition (c = NJ*p + j)
PBAND = NP // G     # 16 partitions per group
EPS = 1e-5
F32 = mybir.dt.float32
F32R = mybir.dt.float32r
F16 = mybir.dt.float16
I32 = mybir.dt.int32
MMCHUNK = 512       # max fp32 moving free dim per matmul (PSUM bank)
PREF = 4            # input prefetch depth (batches)
NR_ITERS = 1        # Newton iterations for rsqrt
MAGIC = 0x5F3759DF  # rsqrt seed constant
# lhsT drops the 1/HW: t = (s-mu)/sqrt(var+eps) is scale-invariant, so use
# raw channel sums as weights and scale eps by HW^2 to match exactly.
EPS_EFF = float(HW) * float(HW) * EPS

# implementation knobs (bisectable)
N_DVE_SUMS = 0      # channel-sum j's done by one DVE reduce (rest: ACT copies)
LHST_ENGINE = "dve"  # "dve" (tensor_scalar) or "act" (Copy w/ scale)
GATE_PSUM = False    # sigmoid writes gate into PSUM (cuts SBUF traffic)
MUL_SPLIT = 2        # j's in the DVE gating mul (rest on GpSimd)

_cache: dict = {}


def _emit(tc, nc, xs, m8, wv, bv, ys):
    AF = mybir.ActivationFunctionType
    OP = mybir.AluOpType
    with ExitStack() as ctx:
        consts = ctx.enter_context(tc.tile_pool(name="consts", bufs=1))
        xpool = ctx.enter_context(tc.tile_pool(name="xpool", bufs=BLOC))
        spool = ctx.enter_context(tc.tile_pool(name="spool", bufs=4))
        lpool = ctx.enter_context(tc.tile_pool(name="lpool", bufs=3))
        cpool = ctx.enter_context(tc.tile_pool(name="cpool", bufs=2))
        gpool = ctx.enter_context(
            tc.tile_pool(name="gpool", bufs=2, space="PSUM")
            if GATE_PSUM
            else tc.tile_pool(name="gpool", bufs=3)
        )
        vpool = ctx.enter_context(tc.tile_pool(name="vpool", bufs=20))
        spsum = ctx.enter_context(tc.tile_pool(name="spsum", bufs=2, space="PSUM"))
        opool = ctx.enter_context(tc.tile_pool(name="opool", bufs=3))

        # M16[p, q] = (p//PBAND == q//PBAND) 0/1 indicator (exact in fp32r);
        # wv/bv are the 16x-replicated per-partition weight/bias columns.
        m16r_sb = consts.tile([NP, NP], F32R)
        nc.sync.dma_start(out=m16r_sb[:], in_=m8[:].bitcast(F32R))
        wv_sb = consts.tile([NP, 1], F32)
        nc.sync.dma_start(out=wv_sb[:], in_=wv[:])
        bv_sb = consts.tile([NP, 1], F32)
        nc.sync.dma_start(out=bv_sb[:], in_=bv[:])
        # dummy sigmoid so ACT's one table load is the sigmoid set (which
        # also holds copy/square) before real work arrives
        warm = consts.tile([NP, 1], F32)
        nc.vector.memset(warm[:], 0.0)
        nc.scalar.activation(out=warm[:], in_=warm[:], func=AF.Sigmoid)

        xts = {}
        state = {}

        def dma_in(b):
            # two halves so phase1 can start on j0/j1 at half-arrival;
            # scalar (ACT) ring: never blocks on xpool with bufs=BLOC
            xt = xpool.tile([NP, NJ, HW], F32)
            nc.scalar.dma_start(out=xt[:, 0:2, :].bitcast(F32R), in_=xs[b, :, 0:2, :])
            nc.scalar.dma_start(out=xt[:, 2:4, :].bitcast(F32R), in_=xs[b, :, 2:4, :])
            xts[b] = xt

        def phase1(b):
            # channel sums + masked lhsT (pre-matmul per-batch prep)
            xt = xts[b]
            sums = spool.tile([NP, NJ], F32, tag="sums")
            if N_DVE_SUMS:
                nc.vector.reduce_sum(
                    out=sums[:, 0:N_DVE_SUMS],
                    in_=xt[:, 0:N_DVE_SUMS, :],
                    axis=mybir.AxisListType.X,
                )
            cps = cpool.tile([NP, HW], F32, tag="cps")
            for j in range(N_DVE_SUMS, NJ):
                nc.scalar.activation(
                    out=cps[:], in_=xt[:, j, :], func=AF.Copy,
                    accum_out=sums[:, j : j + 1],
                )
            # lhsT[p, j, q] = indicator[p, q] * sums[p, j] (masked raw sums)
            lhsT = lpool.tile([NP, NJ, NP], F32R, tag="lhsT")
            for j in range(NJ):
                if LHST_ENGINE == "dve":
                    nc.vector.tensor_scalar_mul(
                        lhsT[:, j, :], m16r_sb[:], sums[:, j : j + 1]
                    )
                else:
                    nc.scalar.activation(
                        out=lhsT[:, j, :], in_=m16r_sb[:].bitcast(F32),
                        func=AF.Copy, scale=sums[:, j : j + 1],
                    )
            state[b] = lhsT

        def phase2(b):
            # s replicated onto each group's 16-partition band (M=128 free)
            xt = xts[b]
            lhsT = state[b]
            ps = spsum.tile([NP, HW], F32)
            for c0 in range(0, HW, MMCHUNK):
                c1 = min(c0 + MMCHUNK, HW)
                for j in range(NJ):
                    nc.tensor.matmul(
                        ps[:, c0:c1], lhsT[:, j, :], xt[:, j, c0:c1].bitcast(F32R),
                        start=(j == 0), stop=(j == NJ - 1),
                    )
            state[b] = ps

        def phase3(b):
            # stats from PSUM + rstd on DVE + gate on ACT
            ps = state[b]
            st6 = vpool.tile([NP, 2, 6], F32, tag="st6")
            nc.vector.bn_stats(st6[:, 0, :], ps[:, 0 : HW // 2])
            nc.vector.bn_stats(st6[:, 1, :], ps[:, HW // 2 : HW])
            mv = vpool.tile([NP, 2], F32, tag="mv")
            nc.vector.bn_aggr(mv[:], st6[:])
            u = vpool.tile([NP, 1], F32, tag="u")
            nc.vector.tensor_scalar_add(u[:], mv[:, 1:2], EPS_EFF)
            # y0 = bits(MAGIC - (bits(u) >> 1)); K - t = (t ^ -1) + (K + 1)
            y = vpool.tile([NP, 1], F32, tag="y")
            yi = y[:].bitcast(I32)
            nc.vector.tensor_scalar(
                out=yi, in0=u[:].bitcast(I32), scalar1=1, scalar2=None,
                op0=OP.logical_shift_right,
            )
            nc.vector.tensor_scalar(
                out=yi, in0=yi, scalar1=-1, scalar2=None, op0=OP.bitwise_xor
            )
            nc.vector.tensor_scalar(
                out=yi, in0=yi, scalar1=MAGIC + 1, scalar2=None, op0=OP.add
            )
            h = vpool.tile([NP, 1], F32, tag="h")
            nc.vector.tensor_scalar_mul(h[:], u[:], -0.5)
            t = vpool.tile([NP, 1], F32, tag="t")
            for _ in range(NR_ITERS):
                nc.vector.tensor_mul(t[:], y[:], y[:])
                nc.vector.tensor_mul(t[:], t[:], h[:])
                nc.vector.scalar_tensor_tensor(
                    out=y[:], in0=t[:], scalar=1.5, in1=y[:],
                    op0=OP.add, op1=OP.mult,
                )
            a_t = vpool.tile([NP, 1], F32, tag="a_t")
            nc.vector.tensor_mul(a_t[:], y[:], wv_sb[:])
            nmu = vpool.tile([NP, 1], F32, tag="nmu")
            nc.vector.tensor_scalar_mul(nmu[:], mv[:, 0:1], -1.0)
            c_t = vpool.tile([NP, 1], F32, tag="c_t")
            nc.vector.scalar_tensor_tensor(
                out=c_t[:], in0=nmu[:], scalar=a_t[:], in1=bv_sb[:],
                op0=OP.mult, op1=OP.add,
            )
            gate = gpool.tile([NP, HW], F32, tag="gate")
            nc.scalar.activation(
                out=gate[:], in_=ps[:, 0:HW], func=AF.Sigmoid, bias=c_t[:],
                scale=a_t[:],
            )
            state[b] = gate

        def phase4(b):
            # gating multiply (fp16 out, 2x DVE fast path) + sync-ring store
            xt = xts.pop(b)
            gate = state.pop(b)
            ot = opool.tile([NP, NJ, HW], F16)
            gb = lambda n: gate[:].unsqueeze(1).to_broadcast([NP, n, HW])
            k = MUL_SPLIT
            nc.vector.tensor_mul(ot[:, 0:k, :], xt[:, 0:k, :], gb(k))
            if k < NJ:
                nc.gpsimd.tensor_mul(
                    ot[:, k:NJ, :], xt[:, k:NJ, :], gb(NJ - k)
                )
                nc.sync.dma_start(out=ys[b, :, 0:k, :], in_=ot[:, 0:k, :])
                nc.sync.dma_start(out=ys[b, :, k:NJ, :], in_=ot[:, k:NJ, :])
            else:
                nc.sync.dma_start(out=ys[b], in_=ot[:])
            if b + PREF < BLOC:
                dma_in(b + PREF)

        # software-pipelined emission: each engine's stream sees work in
        # data-readiness order, so in-order engines never head-of-line block
        for b in range(min(PREF, BLOC)):
            dma_in(b)
        phase1(0)
        phase1(1)
        phase2(0)
        for b in range(BLOC):
            if b + 2 < BLOC:
                phase1(b + 2)
            phase3(b)
            if b + 1 < BLOC:
                phase2(b + 1)
            phase4(b)


def _build_nc():
    nc = bacc.Bacc("TRN2", debug=False)
    xs = nc.dram_tensor("xs", [BLOC, NP, NJ, HW], F32R, kind="ExternalInput")
    m8 = nc.dram_tensor("m8", [NP, NP], F32, kind="ExternalInput")
    wv = nc.dram_tensor("wv", [NP, 1], F32, kind="ExternalInput")
    bv = nc.dram_tensor("bv", [NP, 1], F32, kind="ExternalInput")
    ys = nc.dram_tensor("ys", [BLOC, NP, NJ, HW], F16, kind="ExternalOutput")
    with tile.TileContext(nc) as tc:
        _emit(tc, nc, xs, m8, wv, bv, ys)
    nc.compile()
    return nc


def get_nc():
    if "nc" not in _cache:
        _cache["nc"] = _build_nc()
    return _cache["nc"]


def make_in_maps(x, weight, bias):
    x = np.ascontiguousarray(np.asarray(x, dtype=np.float32))
    weight = np.asarray(weight, dtype=np.float32).reshape(G)
    bias = np.asarray(bias, dtype=np.float32).reshape(G)
    # [core, b, p, j, hw] with c = NJ*p + j
    xs = x.reshape(NCORES, BLOC, NP, NJ, HW)
    band = np.arange(NP) // PBAND
    m8 = (band[:, None] == band[None, :]).astype(np.float32)  # [NP, NP] indicator
    wv = np.ascontiguousarray(np.repeat(weight, PBAND)[:, None])
    bv = np.ascontiguousarray(np.repeat(bias, PBAND)[:, None])
    return [
        {"xs": np.ascontiguousarray(xs[i]), "m8": m8, "wv": wv, "bv": bv}
        for i in range(NCORES)
    ]


def run(x, weight, bias, trace=False, **spmd_kwargs):
    nc = get_nc()
    in_maps = make_in_maps(x, weight, bias)
    res = run_bass_kernel_spmd(
        nc, in_maps, core_ids=list(range(NCORES)), trace=trace, **spmd_kwargs
    )
    out = np.stack(
        [res.results[i]["ys"].astype(np.float32) for i in range(NCORES)]
    )
    return out.reshape(B, C, H, W), res


def kernel(x, weight, bias, groups=G, **_ignored):
    assert int(groups) == G
    out, _ = run(x, weight, bias, trace=False)
    return out


# revision 16
# speedup vs baseline: 1.3132x; 1.3132x over previous
"""Trainium2 Bass kernel: grouped similarity-gating normalization.

Reference computation (per batch b, group g, cpg=64 channels, hw=784):
    means[c]  = mean_hw(x[c, :])
    s[hw]     = sum_c x[c, hw] * means[c]
    t         = (s - mean(s)) * rsqrt(var(s) + eps)
    gate      = sigmoid(t * weight[g] + bias[g])
    out[c,hw] = x[c, hw] * gate[hw]

Sharding: data-parallel over batch B=64 across 8 cores (8 batches/core).

Per-core layout: one SBUF tile [128, 4, 784] per batch holds channels
c = 4*p + j (p = partition, j = free chunk) -> contiguous 1.6MB DMAs, and
group(c) = c//64 = p//16, i.e. each group owns a 16-partition band.

v2 design (memory-regime; HBM floor ~54us/core with fp16 output):
  - input DMAs ride the scalar (ACT) HWDGE ring, output DMAs the sync
    (SP) ring -> no head-of-line blocking between loads and stores.
  - channel sums: j0 via one DVE reduce, j1-3 via ACT Copy+accum_out
    (copy lives in the sigmoid table set -> no ACT table swaps at all).
  - s via PE: 4 accumulating fp32r matmuls with lhsT = indicator*means.
  - mean/var of s via DVE bn_stats/bn_aggr straight from PSUM (exact:
    4 equal-count sub-groups of 196).
  - rstd on DVE: int32 magic-constant seed (shift/xor/add) + 3 Newton
    iterations -> no Sqrt on ACT, so zero ACT_TABLE_LOADs in steady
    state (all ACT funcs sit in sigmoid_and_others).
  - gate = sigmoid(s*a + c) one ACT op with per-partition scale/bias.
  - gating multiply split DVE (j0,j1) / GpSimd (j2,j3), each as ONE
    broadcast-middle-dim tensor_tensor writing fp16 -> output HBM
    traffic halved; host upcasts to fp32 (rel err ~4e-4 << 2e-2 gate).
"""

import sys

if "/opt/trn_rl_repo" not in sys.path:
    sys.path.insert(0, "/opt/trn_rl_repo")

from contextlib import ExitStack

import numpy as np

import concourse.bacc as bacc
import concourse.tile as tile
from concourse import mybir
from concourse.bass_utils import run_bass_kernel_spmd

B, C, H, W = 64, 512, 28, 28
G = 8
HW = H * W          # 784
NCORES = 8
BLOC = B // NCORES  # 8 batches per core
NP = 128            # SBUF partitions
NJ = C // NP        # 4 channel chunks per partition (c = NJ*p + j)
PBAND = NP // G     # 16 partitions per group
EPS = 1e-5
F32 = mybir.dt.float32
F32R = mybir.dt.float32r
F16 = mybir.dt.float16
I32 = mybir.dt.int32
MMCHUNK = 512       # max fp32 moving free dim per matmul (PSUM bank)
PREF = 4            # input prefetch depth (batches)
NR_ITERS = 1        # Newton iterations for rsqrt
MAGIC = 0x5F3759DF  # rsqrt seed constant
# lhsT drops the 1/HW: t = (s-mu)/sqrt(var+eps) is scale-invariant, so use
# raw channel sums as weights and scale eps by HW^2 to match exactly.
EPS_EFF = float(HW) * float(HW) * EPS

# implementation knobs (bisectable)
N_DVE_SUMS = 0      # channel-sum j's done by one DVE reduce (rest: ACT copies)
LHST_ENGINE = "dve"  # "dve" (tensor_scalar) or "act" (Copy w/ scale)
GATE_PSUM = True     # sigmoid writes gate into PSUM (cuts SBUF traffic)
MUL_SPLIT = 4        # j's in the DVE gating mul (rest on GpSimd)

_cache: dict = {}


def _emit(tc, nc, xs, m8, wv, bv, ys):
    AF = mybir.ActivationFunctionType
    OP = mybir.AluOpType
    with ExitStack() as ctx:
        consts = ctx.enter_context(tc.tile_pool(name="consts", bufs=1))
        xpool = ctx.enter_context(tc.tile_pool(name="xpool", bufs=BLOC))
        spool = ctx.enter_context(tc.tile_pool(name="spool", bufs=4))
        lpool = ctx.enter_context(tc.tile_pool(name="lpool", bufs=3))
        cpool = ctx.enter_context(tc.tile_pool(name="cpool", bufs=2))
        gpool = ctx.enter_context(
            tc.tile_pool(name="gpool", bufs=2, space="PSUM")
            if GATE_PSUM
            else tc.tile_pool(name="gpool", bufs=3)
        )
        vpool = ctx.enter_context(tc.tile_pool(name="vpool", bufs=20))
        spsum = ctx.enter_context(tc.tile_pool(name="spsum", bufs=2, space="PSUM"))
        opool = ctx.enter_context(tc.tile_pool(name="opool", bufs=3))

        # M16[p, q] = (p//PBAND == q//PBAND) 0/1 indicator (exact in fp32r);
        # wv/bv are the 16x-replicated per-partition weight/bias columns.
        m16r_sb = consts.tile([NP, NP], F32R)
        nc.sync.dma_start(out=m16r_sb[:], in_=m8[:].bitcast(F32R))
        wv_sb = consts.tile([NP, 1], F32)
        nc.sync.dma_start(out=wv_sb[:], in_=wv[:])
        bv_sb = consts.tile([NP, 1], F32)
        nc.sync.dma_start(out=bv_sb[:], in_=bv[:])
        # dummy sigmoid so ACT's one table load is the sigmoid set (which
        # also holds copy/square) before real work arrives
        warm = consts.tile([NP, 1], F32)
        nc.vector.memset(warm[:], 0.0)
        nc.scalar.activation(out=warm[:], in_=warm[:], func=AF.Sigmoid)

        xts = {}
        state = {}

        def dma_in(b):
            # two halves so phase1 can start on j0/j1 at half-arrival;
            # scalar (ACT) ring: never blocks on xpool with bufs=BLOC.
            # 2 pad columns: col HW collects the channel sums (accum_out)
            # so the matmul's second chunk emits HW*mu for free; col HW+1
            # is zeroed filler keeping fp32r chunk widths even.
            xt = xpool.tile([NP, NJ, HW + 2], F32)
            nc.scalar.dma_start(
                out=xt[:, 0:2, 0:HW].bitcast(F32R), in_=xs[b, :, 0:2, :]
            )
            nc.scalar.dma_start(
                out=xt[:, 2:4, 0:HW].bitcast(F32R), in_=xs[b, :, 2:4, :]
            )
            xts[b] = xt

        def phase1(b):
            # channel sums + masked lhsT (pre-matmul per-batch prep)
            xt = xts[b]
            sums = lambda j: xt[:, j, HW : HW + 1]
            sums_r = lambda j: xt[:, j, HW : HW + 1].bitcast(F32R)
            if N_DVE_SUMS:
                nc.vector.reduce_sum(
                    out=xt[:, 0:N_DVE_SUMS, HW : HW + 1].bitcast(F32R),
                    in_=xt[:, 0:N_DVE_SUMS, 0:HW],
                    axis=mybir.AxisListType.X,
                )
            cps = cpool.tile([NP, HW], F32, tag="cps")
            for j in range(N_DVE_SUMS, NJ):
                with nc.allow_low_precision(
                    reason="accum_out AP is f32r-tagged f32 (raw bits)"
                ):
                    nc.scalar.activation(
                        out=cps[:], in_=xt[:, j, 0:HW], func=AF.Copy,
                        accum_out=sums_r(j),
                    )
            # lhsT[p, j, q] = indicator[p, q] * sums[p, j] (masked raw sums)
            lhsT = lpool.tile([NP, NJ, NP], F32R, tag="lhsT")
            for j in range(NJ):
                if LHST_ENGINE == "dve":
                    nc.vector.tensor_scalar_mul(
                        lhsT[:, j, :], m16r_sb[:], sums(j)
                    )
                else:
                    nc.scalar.activation(
                        out=lhsT[:, j, :], in_=m16r_sb[:].bitcast(F32),
                        func=AF.Copy, scale=sums(j),
                    )
            state[b] = lhsT

        def phase2(b):
            # s replicated onto each group's 16-partition band (M=128 free)
            xt = xts[b]
            lhsT = state[b]
            ps = spsum.tile([NP, HW + 2], F32)
            for c0 in range(0, HW + 2, MMCHUNK):
                c1 = min(c0 + MMCHUNK, HW + 2)
                for j in range(NJ):
                    nc.tensor.matmul(
                        ps[:, c0:c1], lhsT[:, j, :], xt[:, j, c0:c1].bitcast(F32R),
                        start=(j == 0), stop=(j == NJ - 1),
                    )
            state[b] = ps

        def phase3(b):
            # stats: mu free from the matmul's sums column; var via ACT
            # Square+accum (sigmoid-set resident); rstd via DVE int-NR
            ps = state[b]
            nmu = vpool.tile([NP, 1], F32, tag="nmu")
            nc.vector.tensor_scalar_mul(nmu[:], ps[:, HW : HW + 1], -1.0 / HW)
            sq = cpool.tile([NP, HW], F32, tag="sq")
            hwvar = vpool.tile([NP, 1], F32, tag="hwvar")
            nc.scalar.activation(
                out=sq[:], in_=ps[:, 0:HW], func=AF.Square, bias=nmu[:],
                accum_out=hwvar[:],
            )
            u = vpool.tile([NP, 1], F32, tag="u")
            nc.vector.tensor_scalar(
                out=u[:], in0=hwvar[:], scalar1=1.0 / HW, scalar2=EPS_EFF,
                op0=OP.mult, op1=OP.add,
            )
            # y0 = bits(MAGIC - (bits(u) >> 1)); K - t = (t ^ -1) + (K + 1)
            y = vpool.tile([NP, 1], F32, tag="y")
            yi = y[:].bitcast(I32)
            nc.vector.tensor_scalar(
                out=yi, in0=u[:].bitcast(I32), scalar1=1, scalar2=None,
                op0=OP.logical_shift_right,
            )
            nc.vector.tensor_scalar(
                out=yi, in0=yi, scalar1=-1, scalar2=None, op0=OP.bitwise_xor
            )
            nc.vector.tensor_scalar(
                out=yi, in0=yi, scalar1=MAGIC + 1, scalar2=None, op0=OP.add
            )
            h = vpool.tile([NP, 1], F32, tag="h")
            nc.vector.tensor_scalar_mul(h[:], u[:], -0.5)
            t = vpool.tile([NP, 1], F32, tag="t")
            for _ in range(NR_ITERS):
                nc.vector.tensor_mul(t[:], y[:], y[:])
                nc.vector.tensor_mul(t[:], t[:], h[:])
                nc.vector.scalar_tensor_tensor(
                    out=y[:], in0=t[:], scalar=1.5, in1=y[:],
                    op0=OP.add, op1=OP.mult,
                )
            a_t = vpool.tile([NP, 1], F32, tag="a_t")
            nc.vector.tensor_mul(a_t[:], y[:], wv_sb[:])
            c_t = vpool.tile([NP, 1], F32, tag="c_t")
            nc.vector.scalar_tensor_tensor(
                out=c_t[:], in0=nmu[:], scalar=a_t[:], in1=bv_sb[:],
                op0=OP.mult, op1=OP.add,
            )
            gate = gpool.tile([NP, HW], F32, tag="gate")
            nc.scalar.activation(
                out=gate[:], in_=ps[:, 0:HW], func=AF.Sigmoid, bias=c_t[:],
                scale=a_t[:],
            )
            state[b] = gate

        def phase4(b):
            # gating multiply (fp16 out, 2x DVE fast path) + sync-ring store
            xt = xts.pop(b)
            gate = state.pop(b)
            ot = opool.tile([NP, NJ, HW], F16)
            gb = lambda n: gate[:].unsqueeze(1).to_broadcast([NP, n, HW])
            k = MUL_SPLIT
            nc.vector.tensor_mul(ot[:, 0:k, :], xt[:, 0:k, 0:HW], gb(k))
            if k < NJ:
                nc.gpsimd.tensor_mul(
                    ot[:, k:NJ, :], xt[:, k:NJ, 0:HW], gb(NJ - k)
                )
                nc.sync.dma_start(out=ys[b, :, 0:k, :], in_=ot[:, 0:k, :])
                nc.sync.dma_start(out=ys[b, :, k:NJ, :], in_=ot[:, k:NJ, :])
            else:
                nc.sync.dma_start(out=ys[b], in_=ot[:])
            if b + PREF < BLOC:
                dma_in(b + PREF)

        # software-pipelined emission: each engine's stream sees work in
        # data-readiness order, so in-order engines never head-of-line block
        for b in range(min(PREF, BLOC)):
            dma_in(b)
        phase1(0)
        phase1(1)
        phase2(0)
        for b in range(BLOC):
            if b + 2 < BLOC:
                phase1(b + 2)
            phase3(b)
            if b + 1 < BLOC:
                phase2(b + 1)
            phase4(b)


def _build_nc():
    nc = bacc.Bacc("TRN2", debug=False)
    xs = nc.dram_tensor("xs", [BLOC, NP, NJ, HW], F32R, kind="ExternalInput")
    m8 = nc.dram_tensor("m8", [NP, NP], F32, kind="ExternalInput")
    wv = nc.dram_tensor("wv", [NP, 1], F32, kind="ExternalInput")
    bv = nc.dram_tensor("bv", [NP, 1], F32, kind="ExternalInput")
    ys = nc.dram_tensor("ys", [BLOC, NP, NJ, HW], F16, kind="ExternalOutput")
    with tile.TileContext(nc) as tc:
        _emit(tc, nc, xs, m8, wv, bv, ys)
    nc.compile()
    return nc


def get_nc():
    if "nc" not in _cache:
        _cache["nc"] = _build_nc()
    return _cache["nc"]


def make_in_maps(x, weight, bias):
    x = np.ascontiguousarray(np.asarray(x, dtype=np.float32))
    weight = np.asarray(weight, dtype=np.float32).reshape(G)
    bias = np.asarray(bias, dtype=np.float32).reshape(G)
    # [core, b, p, j, hw] with c = NJ*p + j
    xs = x.reshape(NCORES, BLOC, NP, NJ, HW)
    band = np.arange(NP) // PBAND
    m8 = (band[:, None] == band[None, :]).astype(np.float32)  # [NP, NP] indicator
    wv = np.ascontiguousarray(np.repeat(weight, PBAND)[:, None])
    bv = np.ascontiguousarray(np.repeat(bias, PBAND)[:, None])
    return [
        {"xs": np.ascontiguousarray(xs[i]), "m8": m8, "wv": wv, "bv": bv}
        for i in range(NCORES)
    ]


def run(x, weight, bias, trace=False, **spmd_kwargs):
    nc = get_nc()
    in_maps = make_in_maps(x, weight, bias)
    res = run_bass_kernel_spmd(
        nc, in_maps, core_ids=list(range(NCORES)), trace=trace, **spmd_kwargs
    )
    out = np.stack(
        [res.results[i]["ys"].astype(np.float32) for i in range(NCORES)]
    )
    return out.reshape(B, C, H, W), res


def kernel(x, weight, bias, groups=G, **_ignored):
    assert int(groups) == G
    out, _ = run(x, weight, bias, trace=False)
    return out
